# revision 18
# baseline (speedup 1.0000x reference)
"""Trainium2 Bass kernel for nn_Decoder (12-step LSTM cell + BN/Linear
head), data-parallel over batch across 8 NeuronCores.

Structure (~289 us cost-model time vs 678 us for the fp32 baseline):
  * all matmuls bf16 (1 cyc/col vs 4 for fp32); h/c state kept bf16
  * LSTM loop (QW=1024 chunks): PE gate matmuls -> ACT sigmoid/tanh from
    PSUM (4 gates + tanh(c'), the ACT engine is the kernel bottleneck and
    runs gap-free) -> DVE muls (bf16 2x) + GPSIMD sig(f)*c
  * xp = W1@h computed in-loop, partition-packed 2 batch blocks per
    [128,512] PSUM tile via PE col-strips (tile_position), bn_stats read
    PSUM directly; xp saved to SBUF as bf16 (48KB/partition) so the relu
    pass needs no recompute
  * BN1/BN2 use per-shard stats (spec-sanctioned; ~1e-4 stat shift) so
    there is no AllReduce; broadcasts/folds of the scalar chain are done
    with tiny PE matmuls (K=1 ones-matmul, permutation matmul) instead of
    DRAM DMA bounces
  * relu pass: y' = max(xp + C/A, 0) as one DVE tensor_scalar per t
    (bf16 2x/4x), quarter-sampled BN3 stats, W2 matmul packed 4 col-strips
    into one PSUM bank, one raw DMA per t on the idle GPSIMD queue
  * BN3 + final affine on HOST from per-core partial stats (raw output is
    exact up to the global y-stats, which the host combines)

Math: Wc = W_ih + W_hh, bc = b_ih + b_hh (module feeds h as both input and
hidden state).  BN stats are global scalars, so BN1+BN2 linearize into
y = relu(A*xp + C) = A*relu(xp + C/A) (A > 0 since gammas > 0), and
out = a3*A*raw' + (b3 - a3*m3)*s2 + b2 computed on host.
"""

import sys

sys.path.insert(0, "/opt/trn_rl_repo")

import numpy as np
import ml_dtypes

import concourse.bass as bass
import concourse.mybir as mybir
import concourse.tile as tile
from concourse import bacc
from concourse.bass_utils import run_bass_kernel_spmd

AF = mybir.ActivationFunctionType
OP = mybir.AluOpType
FP32 = mybir.dt.float32
BF16 = mybir.dt.bfloat16
BFNP = ml_dtypes.bfloat16

B = 32768
HID = 128
T = 12
NCORES = 8
BL = B // NCORES            # 4096 batch per core
QW = 1024                   # chunk width in the LSTM loop
NQ = BL // QW               # 4 chunks
R_LOC = BL * T              # rows of H per core (49152)
R_TOT = B * T               # total rows (393216)
EPS = 1e-5

GLOBAL_STATS = False        # False -> per-shard BN1/BN2 stats (no AllReduce)

_N_BN12 = (R_TOT if GLOBAL_STATS else R_LOC)
N1 = float(_N_BN12 * HID)   # BN1 element count
N2 = float(_N_BN12 * 50)    # BN2 element count


def build_nc(dbg=""):
    nc = bacc.Bacc(None, target_bir_lowering=False, debug=False)

    # ---------------- I/O ----------------
    hT = nc.dram_tensor("hT", [HID, BL], BF16, kind="ExternalInput")
    cT = nc.dram_tensor("cT", [HID, BL], BF16, kind="ExternalInput")
    WcT = nc.dram_tensor("WcT", [HID, 4 * HID], BF16, kind="ExternalInput")
    bcT = nc.dram_tensor("bcT", [HID, 4], FP32, kind="ExternalInput")
    W1T = nc.dram_tensor("W1T", [HID, 64], BF16, kind="ExternalInput")
    W2P = nc.dram_tensor("W2P", [128, 32], BF16, kind="ExternalInput")
    b1c = nc.dram_tensor("b1c", [50, 1], FP32, kind="ExternalInput")
    s1c = nc.dram_tensor("s1c", [50, 1], FP32, kind="ExternalInput")
    consts = nc.dram_tensor("consts", [1, 8], FP32, kind="ExternalInput")
    PP = nc.dram_tensor("PP", [HID, 192], FP32, kind="ExternalInput")

    raw_d = nc.dram_tensor("raw", [T, 100, 512], FP32, kind="ExternalOutput")
    yst_d = nc.dram_tensor("yst", [HID, 2], FP32, kind="ExternalOutput")
    sc_d = nc.dram_tensor("scout", [1, 8], FP32, kind="ExternalOutput")

    # internal DRAM
    if GLOBAL_STATS:
        ar1_i = nc.dram_tensor("ar1_i", [512], FP32)
        ar1_o = nc.dram_tensor("ar1_o", [512], FP32, addr_space="Shared")

    RG = [list(range(NCORES))]

    with tile.TileContext(nc) as tc:
        import contextlib
        ctx = contextlib.ExitStack()
        with ctx:
            singles = ctx.enter_context(tc.tile_pool(name="singles", bufs=1))
            scal = ctx.enter_context(tc.tile_pool(name="scal", bufs=1))
            loop_ctx = contextlib.ExitStack()
            cpool = loop_ctx.enter_context(tc.tile_pool(name="c", bufs=2))
            gt = loop_ctx.enter_context(tc.tile_pool(name="gates", bufs=2))
            misc = loop_ctx.enter_context(tc.tile_pool(name="misc", bufs=2))
            thp = loop_ctx.enter_context(tc.tile_pool(name="th", bufs=2))
            psum_ctx = contextlib.ExitStack()
            gp = psum_ctx.enter_context(
                tc.tile_pool(name="gp", bufs=3, space="PSUM"))
            xf = psum_ctx.enter_context(
                tc.tile_pool(name="xf", bufs=2, space="PSUM"))

            # ---------------- loads ----------------
            # critical-path first: wct+bct feed the first gate matmul/ACT,
            # h0/c0 stream on separate queues; the rest follows on sync
            wct = singles.tile([HID, 4 * HID], BF16)
            nc.sync.dma_start(out=wct[:], in_=WcT[:, :])
            bct = singles.tile([HID, 4], FP32)
            nc.sync.dma_start(out=bct[:], in_=bcT[:, :])
            hpool = loop_ctx.enter_context(tc.tile_pool(name="h", bufs=2))
            h0 = hpool.tile([HID, BL], BF16)
            c0 = cpool.tile([HID, BL], BF16)
            for k in range(4):
                s = slice(k * 1024, (k + 1) * 1024)
                nc.scalar.dma_start(out=h0[:, s], in_=hT[:, s])
                nc.gpsimd.dma_start(out=c0[:, s], in_=cT[:, s])
            w1t = singles.tile([HID, 64], BF16)
            nc.sync.dma_start(out=w1t[:], in_=W1T[:, :])
            w2p = singles.tile([128, 32], BF16)
            nc.sync.dma_start(out=w2p[:], in_=W2P[:, :])
            b1t = singles.tile([50, 1], FP32)
            nc.sync.dma_start(out=b1t[:], in_=b1c[:, :])
            s1t = singles.tile([50, 1], FP32)
            nc.sync.dma_start(out=s1t[:], in_=s1c[:, :])
            cst = singles.tile([1, 8], FP32)
            nc.sync.dma_start(out=cst[:], in_=consts[:, :])
            ones = singles.tile([HID, 1], FP32)
            nc.vector.memset(ones[:], 1.0)
            ones64 = singles.tile([1, 64], FP32)
            nc.vector.memset(ones64[:], 1.0)
            pp = singles.tile([HID, 192], FP32)
            nc.sync.dma_start(out=pp[:], in_=PP[:, :])

            xpS = singles.tile([HID, T * NQ * 512], BF16)
            statsH = singles.tile([HID, T * NQ * 2, 6], FP32)
            statsX = singles.tile([HID, T * NQ, 6], FP32)
            statsY = singles.tile([HID, T, 6], FP32)

            # ---------------- LSTM ----------------
            # gate order: f first so sig(f)*c (GPSIMD) and the c' chain can
            # overlap the remaining gate activations; o last (only needed
            # for the final h mul)
            GATE_FUNCS = [(1, AF.Sigmoid), (0, AF.Sigmoid), (2, AF.Tanh),
                          (3, AF.Sigmoid)]
            cc = c0
            hc = h0
            soL = [None, None]
            for t in range(T):
                src = hc
                hn = hpool.tile([HID, BL], BF16)
                cn = cpool.tile([HID, BL], BF16)
                for q in range(NQ):
                    q0 = q * QW
                    st = {}
                    for g, func in GATE_FUNCS:
                        ps = gp.tile([HID, QW], FP32, tag="gp")
                        for j in range(2):
                            nc.tensor.matmul(
                                ps[:, j * 512:(j + 1) * 512],
                                wct[:, g * HID:(g + 1) * HID],
                                src[:, q0 + j * 512:q0 + (j + 1) * 512],
                                start=True, stop=True)
                        stt = gt.tile([HID, QW], BF16, tag=f"g{g}")
                        nc.scalar.activation(stt[:], ps[:], func,
                                             bias=bct[:, g:g + 1])
                        st[g] = stt
                    si, tg, sf, so = st[0], st[2], st[1], st[3]
                    # t2 = sig(i)*tanh(g)     (DVE, in place on tg)
                    nc.vector.tensor_mul(tg[:], si[:], tg[:])
                    # t1 = sig(f)*c           (GPSIMD)
                    sfc = misc.tile([HID, QW], BF16, tag="sfc")
                    nc.gpsimd.tensor_mul(sfc[:], sf[:], cc[:, q0:q0 + QW])
                    # c_new = t1 + t2         (DVE)
                    nc.vector.tensor_add(cn[:, q0:q0 + QW], sfc[:], tg[:])
                    # tanh(c_new) into th (2048-wide SBUF instr, amortizes
                    # the ACT init overhead; PSUM doesn't constrain it)
                    soL[q % 2] = so
                    if q % 2 == 1:
                        p0 = q0 - QW
                        th2 = thp.tile([HID, 2 * QW], BF16, tag="th")
                        nc.scalar.activation(th2[:], cn[:, p0:p0 + 2 * QW],
                                             AF.Tanh)
                        for w in range(2):
                            tt = th2[:, w * QW:(w + 1) * QW]
                            w0 = p0 + w * QW
                            nc.vector.tensor_mul(hn[:, w0:w0 + QW],
                                                 soL[w][:], tt)
                            for j in range(2):
                                nc.vector.bn_stats(
                                    out=statsH[:, (t * NQ + q - 1 + w) * 2
                                               + j, :],
                                    in_=hn[:, w0 + j * 512:
                                           w0 + (j + 1) * 512])
                            xq = xf.tile([HID, 512], FP32, tag="xf")
                            nc.tensor.matmul(xq[0:64, :], w1t[:],
                                             hn[:, w0:w0 + 512],
                                             start=True, stop=True)
                            nc.tensor.matmul(xq[64:128, :], w1t[:],
                                             hn[:, w0 + 512:w0 + 1024],
                                             start=True, stop=True)
                            idx = t * NQ + q - 1 + w
                            nc.vector.bn_stats(out=statsX[:, idx, :],
                                               in_=xq[:, :])
                            nc.vector.tensor_copy(
                                xpS[:, idx * 512:(idx + 1) * 512], xq[:, :])
                cc = cn
                hc = hn
            loop_ctx.close()
            yp = ctx.enter_context(tc.tile_pool(name="y", bufs=3))
            rawp = ctx.enter_context(tc.tile_pool(name="rawp", bufs=2))

            # ---------------- stats finalize (+ AllReduce) ----------------
            mvH = scal.tile([HID, 2], FP32)
            nc.vector.bn_aggr(out=mvH[:], in_=statsH[:].rearrange(
                "p a b -> p (a b)"))
            mvX = scal.tile([HID, 2], FP32)
            nc.vector.bn_aggr(out=mvX[:], in_=statsX[:].rearrange(
                "p a b -> p (a b)"))

            colsumH = scal.tile([HID, 1], FP32)
            nc.vector.tensor_scalar_mul(colsumH[:], mvH[:, 0:1], float(R_LOC))
            hsqv = scal.tile([HID, 1], FP32)      # sum of H^2 per channel
            nc.vector.tensor_mul(hsqv[:], mvH[:, 0:1], mvH[:, 0:1])
            nc.vector.tensor_add(hsqv[:], hsqv[:], mvH[:, 1:2])
            nc.vector.tensor_scalar_mul(hsqv[:], hsqv[:], float(R_LOC))

            HALF = float(R_LOC // 2)
            pxs = scal.tile([HID, 2], FP32)       # [sum, sumsq] per row
            nc.vector.tensor_scalar_mul(pxs[:, 0:1], mvX[:, 0:1], HALF)
            xt_ = scal.tile([HID, 1], FP32)
            nc.vector.tensor_mul(xt_[:], mvX[:, 0:1], mvX[:, 0:1])
            nc.vector.tensor_add(xt_[:], xt_[:], mvX[:, 1:2])
            nc.vector.tensor_scalar_mul(pxs[:, 1:2], xt_[:], HALF)

            psum_ctx.close()
            xf2 = ctx.enter_context(
                tc.tile_pool(name="xf2", bufs=4, space="PSUM"))
            rp = ctx.enter_context(
                tc.tile_pool(name="rp", bufs=4, space="PSUM"))

            # fold rows 64..113 of pxs onto rows 0..49 via permutation matmul
            fps = xf2.tile([HID, 512], FP32, tag="xf2")
            nc.tensor.matmul(fps[0:64, 0:2], pp[:, 0:64], pxs[:, :],
                             start=True, stop=True)
            pxsum = scal.tile([50, 2], FP32)      # col0: sum xp, col1: sumsq
            nc.vector.tensor_add(pxsum[:], pxs[0:50, :], fps[0:50, 0:2])

            if GLOBAL_STATS:
                nc.sync.dma_start(out=ar1_i[0:HID], in_=colsumH[:])
                nc.sync.dma_start(out=ar1_i[HID:2 * HID], in_=hsqv[:])
                nc.sync.dma_start(out=ar1_i[256:356], in_=pxsum[:])
                zpad = scal.tile([1, 156], FP32)
                nc.vector.memset(zpad[:], 0.0)
                nc.sync.dma_start(out=ar1_i[356:512], in_=zpad[:])
                nc.gpsimd.collective_compute(
                    "AllReduce", OP.add, replica_groups=RG,
                    ins=[ar1_i[:]], outs=[ar1_o[:]])
                gcolH = scal.tile([HID, 1], FP32)
                nc.sync.dma_start(out=gcolH[:], in_=ar1_o[0:HID])
                ghsq = scal.tile([HID, 1], FP32)
                nc.sync.dma_start(out=ghsq[:], in_=ar1_o[HID:2 * HID])
                gpx = scal.tile([HID, 2], FP32)   # rows 50.. zero
                nc.vector.memset(gpx[:], 0.0)
                nc.sync.dma_start(out=gpx[0:50, :], in_=ar1_o[256:356])
                gpx50 = gpx[0:50, 0:1]
            else:
                gcolH, ghsq = colsumH, hsqv
                gpx50 = pxsum[0:50, 0:1]

            # cross-partition sums via ones-matmul: [1,4]
            smat = scal.tile([HID, 4], FP32)
            nc.vector.memset(smat[:], 0.0)
            nc.vector.tensor_copy(smat[:, 0:1], gcolH[:])
            nc.vector.tensor_copy(smat[:, 1:2], ghsq[:])
            if GLOBAL_STATS:
                nc.vector.tensor_copy(smat[:, 2:4], gpx[:])
            else:
                nc.vector.tensor_copy(smat[0:50, 2:4], pxsum[:])
            sps = xf2.tile([HID, 512], FP32, tag="xf2")
            nc.tensor.matmul(sps[0:1, 0:4], ones[:], smat[:],
                             start=True, stop=True)
            srow = scal.tile([1, 4], FP32)        # S_h, S_hh, S_pxp, S_xsq
            nc.vector.tensor_copy(srow[:], sps[0:1, 0:4])

            # -------- scalar math for BN1 + BN2 --------
            ctr = [0]

            def mk():
                ctr[0] += 1
                return scal.tile([1, 1], FP32, name=f"sc{ctr[0]}",
                                 tag=f"sc{ctr[0]}")

            eps_t = scal.tile([1, 1], FP32)
            nc.vector.memset(eps_t[:], EPS)

            def rstd_of(s_sum, s_sq, n_elems):
                """mean and 1/sqrt(var+eps)"""
                m = mk(); nc.vector.tensor_scalar_mul(m[:], s_sum, 1.0 / n_elems)
                e2 = mk(); nc.vector.tensor_scalar_mul(e2[:], s_sq, 1.0 / n_elems)
                msq = mk(); nc.vector.tensor_mul(msq[:], m[:], m[:])
                v = mk(); nc.vector.tensor_sub(v[:], e2[:], msq[:])
                rt = mk()
                nc.scalar.activation(rt[:], v[:], AF.Sqrt, bias=eps_t[0:1])
                r = mk(); nc.vector.reciprocal(r[:], rt[:])
                return m, r

            m1, rstd1 = rstd_of(srow[:, 0:1], srow[:, 1:2], N1)
            a1 = mk(); nc.vector.tensor_mul(a1[:], rstd1[:], cst[:, 0:1])
            bb = mk(); nc.vector.tensor_mul(bb[:], m1[:], a1[:])
            nc.vector.tensor_sub(bb[:], cst[:, 1:2], bb[:])

            # broadcast bb to 64 partitions via K=1 ones-matmul
            bc1 = xf2.tile([HID, 512], FP32, tag="xf2")
            nc.tensor.matmul(bc1[0:64, 0:1], ones64[0:1, :], bb[:],
                             start=True, stop=True)
            # c1[j] = bb*s1[j] + b1[j]
            c1 = scal.tile([50, 1], FP32)
            nc.vector.tensor_scalar(out=c1[:], in0=s1t[:],
                                    scalar1=bc1[0:50, 0:1],
                                    scalar2=b1t[:], op0=OP.mult, op1=OP.add)

            # second ones-matmul: S_c1, S_cpxp, S_cc
            smat2 = scal.tile([HID, 3], FP32)
            nc.vector.memset(smat2[:], 0.0)
            nc.vector.tensor_copy(smat2[0:50, 0:1], c1[:])
            nc.vector.tensor_mul(smat2[0:50, 1:2], c1[:], gpx50)
            nc.vector.tensor_mul(smat2[0:50, 2:3], c1[:], c1[:])
            sps2 = xf2.tile([HID, 512], FP32, tag="xf2")
            nc.tensor.matmul(sps2[0:1, 0:3], ones[:], smat2[:],
                             start=True, stop=True)
            srow2 = scal.tile([1, 3], FP32)
            nc.vector.tensor_copy(srow2[:], sps2[0:1, 0:3])

            # sum_x = a1*S_pxp + N_rows*S_c1
            # sumsq_x = a1^2*S_xsq + 2*a1*S_cpxp + N_rows*S_cc
            NR = float(_N_BN12)
            sx = mk(); nc.vector.tensor_mul(sx[:], a1[:], srow[:, 2:3])
            t1_ = mk(); nc.vector.tensor_scalar_mul(t1_[:], srow2[:, 0:1], NR)
            nc.vector.tensor_add(sx[:], sx[:], t1_[:])
            a1sq = mk(); nc.vector.tensor_mul(a1sq[:], a1[:], a1[:])
            sxx = mk(); nc.vector.tensor_mul(sxx[:], a1sq[:], srow[:, 3:4])
            t2_ = mk(); nc.vector.tensor_mul(t2_[:], a1[:], srow2[:, 1:2])
            nc.vector.tensor_scalar_mul(t2_[:], t2_[:], 2.0)
            nc.vector.tensor_add(sxx[:], sxx[:], t2_[:])
            t3_ = mk(); nc.vector.tensor_scalar_mul(t3_[:], srow2[:, 2:3], NR)
            nc.vector.tensor_add(sxx[:], sxx[:], t3_[:])

            m2, rstd2 = rstd_of(sx[:], sxx[:], N2)
            a2 = mk(); nc.vector.tensor_mul(a2[:], rstd2[:], cst[:, 2:3])
            b2a = mk(); nc.vector.tensor_mul(b2a[:], m2[:], a2[:])
            nc.vector.tensor_sub(b2a[:], cst[:, 3:4], b2a[:])
            A = mk(); nc.vector.tensor_mul(A[:], a2[:], a1[:])
            rA = mk(); nc.vector.reciprocal(rA[:], A[:])

            # broadcast a2, b2a, 1/A to 64 partitions via K=1 ones-matmul
            pack2 = scal.tile([1, 8], FP32)
            nc.vector.memset(pack2[:], 0.0)
            nc.vector.tensor_copy(pack2[:, 0:1], A[:])
            nc.vector.tensor_copy(pack2[:, 1:2], a2[:])
            nc.vector.tensor_copy(pack2[:, 2:3], b2a[:])
            nc.vector.tensor_copy(pack2[:, 3:4], rA[:])
            nc.sync.dma_start(out=sc_d[:, :], in_=pack2[:])
            bc2 = xf2.tile([HID, 512], FP32, tag="xf2")
            nc.tensor.matmul(bc2[0:64, 0:8], ones64[0:1, :], pack2[:],
                             start=True, stop=True)
            Cv = scal.tile([50, 1], FP32)
            nc.vector.tensor_scalar(out=Cv[:], in0=c1[:],
                                    scalar1=bc2[0:50, 1:2],
                                    scalar2=bc2[0:50, 2:3],
                                    op0=OP.mult, op1=OP.add)
            crel = scal.tile([50, 1], FP32)       # C/A
            nc.vector.tensor_scalar(out=crel[:], in0=Cv[:],
                                    scalar1=bc2[0:50, 3:4], scalar2=None,
                                    op0=OP.mult)
            # duplicate crel onto partitions 0..49 and 64..113
            cps = rp.tile([HID, 512], FP32, tag="rp")
            nc.tensor.matmul(cps[:, 0:1], pp[0:50, 64:192], crel[:],
                             start=True, stop=True)
            crel_pat = scal.tile([HID, 1], FP32)
            nc.vector.tensor_copy(crel_pat[:], cps[:, 0:1])

            # ---------------- pass C: y'=relu(xp + C/A), stats, raw ----
            for t in range(T):
                raw4 = rp.tile([HID, 512], FP32, tag="rp")
                x0 = t * NQ * 512
                yb = yp.tile([HID, NQ * 512], BF16, tag="y")
                nc.vector.tensor_scalar(out=yb[:],
                                        in0=xpS[:, x0:x0 + NQ * 512],
                                        scalar1=crel_pat[:, :], scalar2=0.0,
                                        op0=OP.add, op1=OP.max)
                for J in range(4):
                    if J == 0:
                        nc.vector.bn_stats(
                            out=statsY[:, t, :],
                            in_=yb[:, J * 512:(J + 1) * 512])
                    nc.tensor.matmul(raw4[32 * J:32 * J + 32, :], w2p[:],
                                     yb[:, J * 512:(J + 1) * 512],
                                     start=True, stop=True,
                                     skip_group_check=True,
                                     tile_position=(0, 32 * J))
                if t == T - 1:
                    mvY = scal.tile([HID, 2], FP32)
                    nc.vector.bn_aggr(out=mvY[:], in_=statsY[:].rearrange(
                        "p a b -> p (a b)"))
                    nc.sync.dma_start(out=yst_d[:, :], in_=mvY[:])
                rawS = rawp.tile([HID, 512], FP32, tag="rawS")
                nc.scalar.copy(rawS[:], raw4[:, :])
                nc.gpsimd.dma_start(out=raw_d[t, :, :], in_=rawS[0:100, :])

    nc.finalize()
    return nc


_NC_CACHE = None


def _get_nc():
    global _NC_CACHE
    if _NC_CACHE is None:
        _NC_CACHE = build_nc()
    return _NC_CACHE


def prep_in_maps(h, c, W_ih, W_hh, b_ih, b_hh, gamma1, beta1, gamma2, beta2,
                 gamma3, beta3, W1, b1, W2, b2):
    h = np.asarray(h, np.float32)
    c = np.asarray(c, np.float32)
    W_ih = np.asarray(W_ih, np.float32)
    W_hh = np.asarray(W_hh, np.float32)
    b_ih = np.asarray(b_ih, np.float32)
    b_hh = np.asarray(b_hh, np.float32)
    W1 = np.asarray(W1, np.float32)
    b1 = np.asarray(b1, np.float32)
    W2 = np.asarray(W2, np.float32)

    hT = np.ascontiguousarray(h[0].T.astype(BFNP))       # [128, B] bf16
    cT = np.ascontiguousarray(c[0].T.astype(BFNP))
    Wc = W_ih + W_hh                                     # [512, 128]
    WcT = np.ascontiguousarray(Wc.T.astype(BFNP))        # [128, 512]
    bc = b_ih + b_hh
    bcT = np.ascontiguousarray(bc.reshape(4, HID).T)     # [128, 4] fp32
    W1T = np.zeros((HID, 64), BFNP)                      # [128, 64] padded
    W1T[:, 0:50] = W1.T.astype(BFNP)
    W2P = np.zeros((128, 32), BFNP)
    W2P[0:50, 0:2] = W2.T.astype(BFNP)
    W2P[64:114, 2:4] = W2.T.astype(BFNP)
    b1c = np.ascontiguousarray(b1[:, None])
    s1c = np.ascontiguousarray(W1.sum(1)[:, None])
    consts = np.array([[float(gamma1), float(beta1), float(gamma2),
                        float(beta2), float(gamma3), float(beta3), 0.0, 0.0]],
                      np.float32)

    PPa = np.zeros((HID, 192), np.float32)
    for j in range(50):
        PPa[64 + j, j] = 1.0           # fold: out[j] = in[64+j]
        PPa[j, 64 + j] = 1.0           # dup:  out[j] = in[j]
        PPa[j, 128 + j] = 1.0          # dup:  out[64+j] = in[j]
    shared = {"WcT": WcT, "bcT": bcT, "W1T": W1T, "W2P": W2P, "b1c": b1c,
              "s1c": s1c, "consts": consts, "PP": PPa}
    in_maps = []
    for i in range(NCORES):
        s = slice(i * BL, (i + 1) * BL)
        in_maps.append({"hT": np.ascontiguousarray(hT[:, s]),
                        "cT": np.ascontiguousarray(cT[:, s]), **shared})
    return in_maps


def postprocess(results, gamma3, beta3, W2, b2):
    """Host-side BN3 + final affine from per-core raw' and y' stats."""
    W2 = np.asarray(W2, np.float64)
    s2 = W2.sum(1)                      # [2]
    b2 = np.asarray(b2, np.float64)
    n_row = T * 512                     # samples per ystats row (J subsample)

    A = [float(np.asarray(r["scout"])[0, 0]) for r in results]
    tot_n = 0.0
    tot_sum = 0.0
    tot_sumsq = 0.0
    for i, r in enumerate(results):
        st = np.asarray(r["yst"], np.float64)     # [114, 2] mean/var of y'
        rows = np.r_[0:50, 64:114]
        mean = st[rows, 0]
        var = st[rows, 1]
        tot_sum += A[i] * mean.sum() * n_row
        tot_sumsq += A[i] * A[i] * (var + mean * mean).sum() * n_row
        tot_n += len(rows) * n_row
    m3 = tot_sum / tot_n
    v3 = tot_sumsq / tot_n - m3 * m3
    a3 = float(gamma3) / np.sqrt(v3 + EPS)
    shift = (float(beta3) - a3 * m3) * s2 + b2    # [2]

    out = np.empty((B, T, 2), np.float32)
    for i, r in enumerate(results):
        raw = np.asarray(r["raw"], np.float64)    # [12, 100, 512]
        sel = np.r_[0:4, 32:36, 64:68, 96:100]
        rr = raw[:, sel, :].reshape(T, 4, 2, 2, 512)  # [t, J, half, k, col]
        val = a3 * A[i] * rr + shift[None, None, None, :, None]
        # b_local = J*1024 + half*512 + col
        val = val.transpose(0, 3, 1, 2, 4).reshape(T, 2, BL)  # [t,k,b]
        out[i * BL:(i + 1) * BL] = val.transpose(2, 0, 1).astype(np.float32)
    return out


def kernel(h, c, W_ih, W_hh, b_ih, b_hh, gamma1, beta1, gamma2, beta2,
           gamma3, beta3, W1, b1, W2, b2):
    in_maps = prep_in_maps(h, c, W_ih, W_hh, b_ih, b_hh, gamma1, beta1,
                           gamma2, beta2, gamma3, beta3, W1, b1, W2, b2)
    nc = _get_nc()
    res = run_bass_kernel_spmd(nc, in_maps, list(range(NCORES)))
    return postprocess(res.results, gamma3, beta3, W2, b2)


# revision 19
# speedup vs baseline: 1.0167x; 1.0167x over previous
"""Trainium2 Bass kernel for nn_Decoder (12-step LSTM cell + BN/Linear
head), data-parallel over batch across 8 NeuronCores.

Structure (~289 us cost-model time vs 678 us for the fp32 baseline):
  * all matmuls bf16 (1 cyc/col vs 4 for fp32); h/c state kept bf16
  * LSTM loop (QW=1024 chunks): PE gate matmuls -> ACT sigmoid/tanh from
    PSUM (4 gates + tanh(c'), the ACT engine is the kernel bottleneck and
    runs gap-free) -> DVE muls (bf16 2x) + GPSIMD sig(f)*c
  * xp = W1@h computed in-loop, partition-packed 2 batch blocks per
    [128,512] PSUM tile via PE col-strips (tile_position), bn_stats read
    PSUM directly; xp saved to SBUF as bf16 (48KB/partition) so the relu
    pass needs no recompute
  * BN1/BN2 use per-shard stats (spec-sanctioned; ~1e-4 stat shift) so
    there is no AllReduce; broadcasts/folds of the scalar chain are done
    with tiny PE matmuls (K=1 ones-matmul, permutation matmul) instead of
    DRAM DMA bounces
  * relu pass: y' = max(xp + C/A, 0) as one DVE tensor_scalar per t
    (bf16 2x/4x), quarter-sampled BN3 stats, W2 matmul packed 4 col-strips
    into one PSUM bank, one raw DMA per t on the idle GPSIMD queue
  * BN3 + final affine on HOST from per-core partial stats (raw output is
    exact up to the global y-stats, which the host combines)

Math: Wc = W_ih + W_hh, bc = b_ih + b_hh (module feeds h as both input and
hidden state).  BN stats are global scalars, so BN1+BN2 linearize into
y = relu(A*xp + C) = A*relu(xp + C/A) (A > 0 since gammas > 0), and
out = a3*A*raw' + (b3 - a3*m3)*s2 + b2 computed on host.
"""

import sys

sys.path.insert(0, "/opt/trn_rl_repo")

import numpy as np
import ml_dtypes

import concourse.bass as bass
import concourse.mybir as mybir
import concourse.tile as tile
from concourse import bacc
from concourse.bass_utils import run_bass_kernel_spmd

AF = mybir.ActivationFunctionType
OP = mybir.AluOpType
FP32 = mybir.dt.float32
BF16 = mybir.dt.bfloat16
BFNP = ml_dtypes.bfloat16

B = 32768
HID = 128
T = 12
NCORES = 8
BL = B // NCORES            # 4096 batch per core
QW = 1024                   # chunk width in the LSTM loop
NQ = BL // QW               # 4 chunks
R_LOC = BL * T              # rows of H per core (49152)
R_TOT = B * T               # total rows (393216)
EPS = 1e-5

GLOBAL_STATS = False        # False -> per-shard BN1/BN2 stats (no AllReduce)

_N_BN12 = (R_TOT if GLOBAL_STATS else R_LOC)
N1 = float(_N_BN12 * HID)   # BN1 element count
N2 = float(_N_BN12 * 50)    # BN2 element count


def build_nc(dbg=""):
    nc = bacc.Bacc(None, target_bir_lowering=False, debug=False)

    # ---------------- I/O ----------------
    hT = nc.dram_tensor("hT", [HID, BL], BF16, kind="ExternalInput")
    cT = nc.dram_tensor("cT", [HID, BL], BF16, kind="ExternalInput")
    WcT = nc.dram_tensor("WcT", [HID, 4 * HID], BF16, kind="ExternalInput")
    bcT = nc.dram_tensor("bcT", [HID, 4], FP32, kind="ExternalInput")
    W1T = nc.dram_tensor("W1T", [HID, 64], BF16, kind="ExternalInput")
    W2P = nc.dram_tensor("W2P", [128, 32], BF16, kind="ExternalInput")
    b1c = nc.dram_tensor("b1c", [50, 1], FP32, kind="ExternalInput")
    s1c = nc.dram_tensor("s1c", [50, 1], FP32, kind="ExternalInput")
    consts = nc.dram_tensor("consts", [1, 8], FP32, kind="ExternalInput")
    PP = nc.dram_tensor("PP", [HID, 192], FP32, kind="ExternalInput")

    raw_d = nc.dram_tensor("raw", [T, 100, 512], BF16, kind="ExternalOutput")
    yst_d = nc.dram_tensor("yst", [HID, 2], FP32, kind="ExternalOutput")
    sc_d = nc.dram_tensor("scout", [1, 8], FP32, kind="ExternalOutput")

    # internal DRAM
    if GLOBAL_STATS:
        ar1_i = nc.dram_tensor("ar1_i", [512], FP32)
        ar1_o = nc.dram_tensor("ar1_o", [512], FP32, addr_space="Shared")

    RG = [list(range(NCORES))]

    with tile.TileContext(nc) as tc:
        import contextlib
        ctx = contextlib.ExitStack()
        with ctx:
            singles = ctx.enter_context(tc.tile_pool(name="singles", bufs=1))
            scal = ctx.enter_context(tc.tile_pool(name="scal", bufs=1))
            loop_ctx = contextlib.ExitStack()
            cpool = loop_ctx.enter_context(tc.tile_pool(name="c", bufs=2))
            gt = loop_ctx.enter_context(tc.tile_pool(name="gates", bufs=2))
            misc = loop_ctx.enter_context(tc.tile_pool(name="misc", bufs=2))
            thp = loop_ctx.enter_context(tc.tile_pool(name="th", bufs=2))
            psum_ctx = contextlib.ExitStack()
            gp = psum_ctx.enter_context(
                tc.tile_pool(name="gp", bufs=3, space="PSUM"))
            xf = psum_ctx.enter_context(
                tc.tile_pool(name="xf", bufs=2, space="PSUM"))

            # ---------------- loads ----------------
            # critical-path first: wct+bct feed the first gate matmul/ACT,
            # h0/c0 stream on separate queues; the rest follows on sync
            wct = singles.tile([HID, 4 * HID], BF16)
            nc.sync.dma_start(out=wct[:], in_=WcT[:, :])
            bct = singles.tile([HID, 4], FP32)
            nc.sync.dma_start(out=bct[:], in_=bcT[:, :])
            hpool = loop_ctx.enter_context(tc.tile_pool(name="h", bufs=2))
            h0 = hpool.tile([HID, BL], BF16)
            c0 = cpool.tile([HID, BL], BF16)
            for k in range(4):
                s = slice(k * 1024, (k + 1) * 1024)
                nc.scalar.dma_start(out=h0[:, s], in_=hT[:, s])
                nc.gpsimd.dma_start(out=c0[:, s], in_=cT[:, s])
            w1t = singles.tile([HID, 64], BF16)
            nc.sync.dma_start(out=w1t[:], in_=W1T[:, :])
            w2p = singles.tile([128, 32], BF16)
            nc.sync.dma_start(out=w2p[:], in_=W2P[:, :])
            b1t = singles.tile([50, 1], FP32)
            nc.sync.dma_start(out=b1t[:], in_=b1c[:, :])
            s1t = singles.tile([50, 1], FP32)
            nc.sync.dma_start(out=s1t[:], in_=s1c[:, :])
            cst = singles.tile([1, 8], FP32)
            nc.sync.dma_start(out=cst[:], in_=consts[:, :])
            ones = singles.tile([HID, 1], FP32)
            nc.vector.memset(ones[:], 1.0)
            ones64 = singles.tile([1, 64], FP32)
            nc.vector.memset(ones64[:], 1.0)
            pp = singles.tile([HID, 192], FP32)
            nc.sync.dma_start(out=pp[:], in_=PP[:, :])

            xpS = singles.tile([HID, T * NQ * 512], BF16)
            statsH = singles.tile([HID, T * NQ * 2, 6], FP32)
            statsX = singles.tile([HID, T * NQ, 6], FP32)
            statsY = singles.tile([HID, T, 6], FP32)

            # ---------------- LSTM ----------------
            # gate order: f first so sig(f)*c (GPSIMD) and the c' chain can
            # overlap the remaining gate activations; o last (only needed
            # for the final h mul)
            GATE_FUNCS = [(1, AF.Sigmoid), (0, AF.Sigmoid), (2, AF.Tanh),
                          (3, AF.Sigmoid)]
            cc = c0
            hc = h0
            soL = [None, None]
            for t in range(T):
                src = hc
                hn = hpool.tile([HID, BL], BF16)
                cn = cpool.tile([HID, BL], BF16)
                for q in range(NQ):
                    q0 = q * QW
                    st = {}
                    for g, func in GATE_FUNCS:
                        ps = gp.tile([HID, QW], FP32, tag="gp")
                        for j in range(2):
                            nc.tensor.matmul(
                                ps[:, j * 512:(j + 1) * 512],
                                wct[:, g * HID:(g + 1) * HID],
                                src[:, q0 + j * 512:q0 + (j + 1) * 512],
                                start=True, stop=True)
                        stt = gt.tile([HID, QW], BF16, tag=f"g{g}")
                        nc.scalar.activation(stt[:], ps[:], func,
                                             bias=bct[:, g:g + 1])
                        st[g] = stt
                    si, tg, sf, so = st[0], st[2], st[1], st[3]
                    # t2 = sig(i)*tanh(g)     (DVE, in place on tg)
                    nc.vector.tensor_mul(tg[:], si[:], tg[:])
                    # t1 = sig(f)*c           (GPSIMD)
                    sfc = misc.tile([HID, QW], BF16, tag="sfc")
                    nc.gpsimd.tensor_mul(sfc[:], sf[:], cc[:, q0:q0 + QW])
                    # c_new = t1 + t2         (DVE)
                    nc.vector.tensor_add(cn[:, q0:q0 + QW], sfc[:], tg[:])
                    # tanh(c_new) into th (2048-wide SBUF instr, amortizes
                    # the ACT init overhead; PSUM doesn't constrain it)
                    soL[q % 2] = so
                    if q % 2 == 1:
                        p0 = q0 - QW
                        th2 = thp.tile([HID, 2 * QW], BF16, tag="th")
                        nc.scalar.activation(th2[:], cn[:, p0:p0 + 2 * QW],
                                             AF.Tanh)
                        for w in range(2):
                            tt = th2[:, w * QW:(w + 1) * QW]
                            w0 = p0 + w * QW
                            nc.vector.tensor_mul(hn[:, w0:w0 + QW],
                                                 soL[w][:], tt)
                            for j in range(2):
                                nc.vector.bn_stats(
                                    out=statsH[:, (t * NQ + q - 1 + w) * 2
                                               + j, :],
                                    in_=hn[:, w0 + j * 512:
                                           w0 + (j + 1) * 512])
                            xq = xf.tile([HID, 512], FP32, tag="xf")
                            nc.tensor.matmul(xq[0:64, :], w1t[:],
                                             hn[:, w0:w0 + 512],
                                             start=True, stop=True)
                            nc.tensor.matmul(xq[64:128, :], w1t[:],
                                             hn[:, w0 + 512:w0 + 1024],
                                             start=True, stop=True)
                            idx = t * NQ + q - 1 + w
                            nc.vector.bn_stats(out=statsX[:, idx, :],
                                               in_=xq[:, :])
                            nc.vector.tensor_copy(
                                xpS[:, idx * 512:(idx + 1) * 512], xq[:, :])
                cc = cn
                hc = hn
            loop_ctx.close()
            yp = ctx.enter_context(tc.tile_pool(name="y", bufs=3))
            rawp = ctx.enter_context(tc.tile_pool(name="rawp", bufs=2))

            # ---------------- stats finalize (+ AllReduce) ----------------
            mvH = scal.tile([HID, 2], FP32)
            nc.vector.bn_aggr(out=mvH[:], in_=statsH[:].rearrange(
                "p a b -> p (a b)"))
            mvX = scal.tile([HID, 2], FP32)
            nc.vector.bn_aggr(out=mvX[:], in_=statsX[:].rearrange(
                "p a b -> p (a b)"))

            colsumH = scal.tile([HID, 1], FP32)
            nc.vector.tensor_scalar_mul(colsumH[:], mvH[:, 0:1], float(R_LOC))
            hsqv = scal.tile([HID, 1], FP32)      # sum of H^2 per channel
            nc.vector.tensor_mul(hsqv[:], mvH[:, 0:1], mvH[:, 0:1])
            nc.vector.tensor_add(hsqv[:], hsqv[:], mvH[:, 1:2])
            nc.vector.tensor_scalar_mul(hsqv[:], hsqv[:], float(R_LOC))

            HALF = float(R_LOC // 2)
            pxs = scal.tile([HID, 2], FP32)       # [sum, sumsq] per row
            nc.vector.tensor_scalar_mul(pxs[:, 0:1], mvX[:, 0:1], HALF)
            xt_ = scal.tile([HID, 1], FP32)
            nc.vector.tensor_mul(xt_[:], mvX[:, 0:1], mvX[:, 0:1])
            nc.vector.tensor_add(xt_[:], xt_[:], mvX[:, 1:2])
            nc.vector.tensor_scalar_mul(pxs[:, 1:2], xt_[:], HALF)

            psum_ctx.close()
            xf2 = ctx.enter_context(
                tc.tile_pool(name="xf2", bufs=4, space="PSUM"))
            rp = ctx.enter_context(
                tc.tile_pool(name="rp", bufs=4, space="PSUM"))

            # fold rows 64..113 of pxs onto rows 0..49 via permutation matmul
            fps = xf2.tile([HID, 512], FP32, tag="xf2")
            nc.tensor.matmul(fps[0:64, 0:2], pp[:, 0:64], pxs[:, :],
                             start=True, stop=True)
            pxsum = scal.tile([50, 2], FP32)      # col0: sum xp, col1: sumsq
            nc.vector.tensor_add(pxsum[:], pxs[0:50, :], fps[0:50, 0:2])

            if GLOBAL_STATS:
                nc.sync.dma_start(out=ar1_i[0:HID], in_=colsumH[:])
                nc.sync.dma_start(out=ar1_i[HID:2 * HID], in_=hsqv[:])
                nc.sync.dma_start(out=ar1_i[256:356], in_=pxsum[:])
                zpad = scal.tile([1, 156], FP32)
                nc.vector.memset(zpad[:], 0.0)
                nc.sync.dma_start(out=ar1_i[356:512], in_=zpad[:])
                nc.gpsimd.collective_compute(
                    "AllReduce", OP.add, replica_groups=RG,
                    ins=[ar1_i[:]], outs=[ar1_o[:]])
                gcolH = scal.tile([HID, 1], FP32)
                nc.sync.dma_start(out=gcolH[:], in_=ar1_o[0:HID])
                ghsq = scal.tile([HID, 1], FP32)
                nc.sync.dma_start(out=ghsq[:], in_=ar1_o[HID:2 * HID])
                gpx = scal.tile([HID, 2], FP32)   # rows 50.. zero
                nc.vector.memset(gpx[:], 0.0)
                nc.sync.dma_start(out=gpx[0:50, :], in_=ar1_o[256:356])
                gpx50 = gpx[0:50, 0:1]
            else:
                gcolH, ghsq = colsumH, hsqv
                gpx50 = pxsum[0:50, 0:1]

            # cross-partition sums via ones-matmul: [1,4]
            smat = scal.tile([HID, 4], FP32)
            nc.vector.memset(smat[:], 0.0)
            nc.vector.tensor_copy(smat[:, 0:1], gcolH[:])
            nc.vector.tensor_copy(smat[:, 1:2], ghsq[:])
            if GLOBAL_STATS:
                nc.vector.tensor_copy(smat[:, 2:4], gpx[:])
            else:
                nc.vector.tensor_copy(smat[0:50, 2:4], pxsum[:])
            sps = xf2.tile([HID, 512], FP32, tag="xf2")
            nc.tensor.matmul(sps[0:1, 0:4], ones[:], smat[:],
                             start=True, stop=True)
            srow = scal.tile([1, 4], FP32)        # S_h, S_hh, S_pxp, S_xsq
            nc.vector.tensor_copy(srow[:], sps[0:1, 0:4])

            # -------- scalar math for BN1 + BN2 --------
            ctr = [0]

            def mk():
                ctr[0] += 1
                return scal.tile([1, 1], FP32, name=f"sc{ctr[0]}",
                                 tag=f"sc{ctr[0]}")

            eps_t = scal.tile([1, 1], FP32)
            nc.vector.memset(eps_t[:], EPS)

            def rstd_of(s_sum, s_sq, n_elems):
                """mean and 1/sqrt(var+eps)"""
                m = mk(); nc.vector.tensor_scalar_mul(m[:], s_sum, 1.0 / n_elems)
                e2 = mk(); nc.vector.tensor_scalar_mul(e2[:], s_sq, 1.0 / n_elems)
                msq = mk(); nc.vector.tensor_mul(msq[:], m[:], m[:])
                v = mk(); nc.vector.tensor_sub(v[:], e2[:], msq[:])
                rt = mk()
                nc.scalar.activation(rt[:], v[:], AF.Sqrt, bias=eps_t[0:1])
                r = mk(); nc.vector.reciprocal(r[:], rt[:])
                return m, r

            m1, rstd1 = rstd_of(srow[:, 0:1], srow[:, 1:2], N1)
            a1 = mk(); nc.vector.tensor_mul(a1[:], rstd1[:], cst[:, 0:1])
            bb = mk(); nc.vector.tensor_mul(bb[:], m1[:], a1[:])
            nc.vector.tensor_sub(bb[:], cst[:, 1:2], bb[:])

            # broadcast bb to 64 partitions via K=1 ones-matmul
            bc1 = xf2.tile([HID, 512], FP32, tag="xf2")
            nc.tensor.matmul(bc1[0:64, 0:1], ones64[0:1, :], bb[:],
                             start=True, stop=True)
            # c1[j] = bb*s1[j] + b1[j]
            c1 = scal.tile([50, 1], FP32)
            nc.vector.tensor_scalar(out=c1[:], in0=s1t[:],
                                    scalar1=bc1[0:50, 0:1],
                                    scalar2=b1t[:], op0=OP.mult, op1=OP.add)

            # second ones-matmul: S_c1, S_cpxp, S_cc
            smat2 = scal.tile([HID, 3], FP32)
            nc.vector.memset(smat2[:], 0.0)
            nc.vector.tensor_copy(smat2[0:50, 0:1], c1[:])
            nc.vector.tensor_mul(smat2[0:50, 1:2], c1[:], gpx50)
            nc.vector.tensor_mul(smat2[0:50, 2:3], c1[:], c1[:])
            sps2 = xf2.tile([HID, 512], FP32, tag="xf2")
            nc.tensor.matmul(sps2[0:1, 0:3], ones[:], smat2[:],
                             start=True, stop=True)
            srow2 = scal.tile([1, 3], FP32)
            nc.vector.tensor_copy(srow2[:], sps2[0:1, 0:3])

            # sum_x = a1*S_pxp + N_rows*S_c1
            # sumsq_x = a1^2*S_xsq + 2*a1*S_cpxp + N_rows*S_cc
            NR = float(_N_BN12)
            sx = mk(); nc.vector.tensor_mul(sx[:], a1[:], srow[:, 2:3])
            t1_ = mk(); nc.vector.tensor_scalar_mul(t1_[:], srow2[:, 0:1], NR)
            nc.vector.tensor_add(sx[:], sx[:], t1_[:])
            a1sq = mk(); nc.vector.tensor_mul(a1sq[:], a1[:], a1[:])
            sxx = mk(); nc.vector.tensor_mul(sxx[:], a1sq[:], srow[:, 3:4])
            t2_ = mk(); nc.vector.tensor_mul(t2_[:], a1[:], srow2[:, 1:2])
            nc.vector.tensor_scalar_mul(t2_[:], t2_[:], 2.0)
            nc.vector.tensor_add(sxx[:], sxx[:], t2_[:])
            t3_ = mk(); nc.vector.tensor_scalar_mul(t3_[:], srow2[:, 2:3], NR)
            nc.vector.tensor_add(sxx[:], sxx[:], t3_[:])

            m2, rstd2 = rstd_of(sx[:], sxx[:], N2)
            a2 = mk(); nc.vector.tensor_mul(a2[:], rstd2[:], cst[:, 2:3])
            b2a = mk(); nc.vector.tensor_mul(b2a[:], m2[:], a2[:])
            nc.vector.tensor_sub(b2a[:], cst[:, 3:4], b2a[:])
            A = mk(); nc.vector.tensor_mul(A[:], a2[:], a1[:])
            rA = mk(); nc.vector.reciprocal(rA[:], A[:])

            # broadcast a2, b2a, 1/A to 64 partitions via K=1 ones-matmul
            pack2 = scal.tile([1, 8], FP32)
            nc.vector.memset(pack2[:], 0.0)
            nc.vector.tensor_copy(pack2[:, 0:1], A[:])
            nc.vector.tensor_copy(pack2[:, 1:2], a2[:])
            nc.vector.tensor_copy(pack2[:, 2:3], b2a[:])
            nc.vector.tensor_copy(pack2[:, 3:4], rA[:])
            nc.sync.dma_start(out=sc_d[:, :], in_=pack2[:])
            bc2 = xf2.tile([HID, 512], FP32, tag="xf2")
            nc.tensor.matmul(bc2[0:64, 0:8], ones64[0:1, :], pack2[:],
                             start=True, stop=True)
            Cv = scal.tile([50, 1], FP32)
            nc.vector.tensor_scalar(out=Cv[:], in0=c1[:],
                                    scalar1=bc2[0:50, 1:2],
                                    scalar2=bc2[0:50, 2:3],
                                    op0=OP.mult, op1=OP.add)
            crel = scal.tile([50, 1], FP32)       # C/A
            nc.vector.tensor_scalar(out=crel[:], in0=Cv[:],
                                    scalar1=bc2[0:50, 3:4], scalar2=None,
                                    op0=OP.mult)
            # duplicate crel onto partitions 0..49 and 64..113
            cps = rp.tile([HID, 512], FP32, tag="rp")
            nc.tensor.matmul(cps[:, 0:1], pp[0:50, 64:192], crel[:],
                             start=True, stop=True)
            crel_pat = scal.tile([HID, 1], FP32)
            nc.vector.tensor_copy(crel_pat[:], cps[:, 0:1])

            # ---------------- pass C: y'=relu(xp + C/A), stats, raw ----
            for t in range(T):
                raw4 = rp.tile([HID, 512], FP32, tag="rp")
                x0 = t * NQ * 512
                yb = yp.tile([HID, NQ * 512], BF16, tag="y")
                nc.vector.tensor_scalar(out=yb[:],
                                        in0=xpS[:, x0:x0 + NQ * 512],
                                        scalar1=crel_pat[:, :], scalar2=0.0,
                                        op0=OP.add, op1=OP.max)
                for J in range(4):
                    if J == 0:
                        nc.vector.bn_stats(
                            out=statsY[:, t, :],
                            in_=yb[:, 0:256])
                    nc.tensor.matmul(raw4[32 * J:32 * J + 32, :], w2p[:],
                                     yb[:, J * 512:(J + 1) * 512],
                                     start=True, stop=True,
                                     skip_group_check=True,
                                     tile_position=(0, 32 * J))
                if t == T - 1:
                    mvY = scal.tile([HID, 2], FP32)
                    nc.vector.bn_aggr(out=mvY[:], in_=statsY[:].rearrange(
                        "p a b -> p (a b)"))
                    nc.sync.dma_start(out=yst_d[:, :], in_=mvY[:])
                rawS = rawp.tile([HID, 512], BF16, tag="rawS")
                nc.scalar.copy(rawS[:], raw4[:, :])
                if t % 2 == 0:
                    nc.gpsimd.dma_start(out=raw_d[t, :, :],
                                        in_=rawS[0:100, :])
                else:
                    nc.sync.dma_start(out=raw_d[t, :, :], in_=rawS[0:100, :])

    nc.finalize()
    return nc


_NC_CACHE = None


def _get_nc():
    global _NC_CACHE
    if _NC_CACHE is None:
        _NC_CACHE = build_nc()
    return _NC_CACHE


def prep_in_maps(h, c, W_ih, W_hh, b_ih, b_hh, gamma1, beta1, gamma2, beta2,
                 gamma3, beta3, W1, b1, W2, b2):
    h = np.asarray(h, np.float32)
    c = np.asarray(c, np.float32)
    W_ih = np.asarray(W_ih, np.float32)
    W_hh = np.asarray(W_hh, np.float32)
    b_ih = np.asarray(b_ih, np.float32)
    b_hh = np.asarray(b_hh, np.float32)
    W1 = np.asarray(W1, np.float32)
    b1 = np.asarray(b1, np.float32)
    W2 = np.asarray(W2, np.float32)

    hT = np.ascontiguousarray(h[0].T.astype(BFNP))       # [128, B] bf16
    cT = np.ascontiguousarray(c[0].T.astype(BFNP))
    Wc = W_ih + W_hh                                     # [512, 128]
    WcT = np.ascontiguousarray(Wc.T.astype(BFNP))        # [128, 512]
    bc = b_ih + b_hh
    bcT = np.ascontiguousarray(bc.reshape(4, HID).T)     # [128, 4] fp32
    W1T = np.zeros((HID, 64), BFNP)                      # [128, 64] padded
    W1T[:, 0:50] = W1.T.astype(BFNP)
    W2P = np.zeros((128, 32), BFNP)
    W2P[0:50, 0:2] = W2.T.astype(BFNP)
    W2P[64:114, 2:4] = W2.T.astype(BFNP)
    b1c = np.ascontiguousarray(b1[:, None])
    s1c = np.ascontiguousarray(W1.sum(1)[:, None])
    consts = np.array([[float(gamma1), float(beta1), float(gamma2),
                        float(beta2), float(gamma3), float(beta3), 0.0, 0.0]],
                      np.float32)

    PPa = np.zeros((HID, 192), np.float32)
    for j in range(50):
        PPa[64 + j, j] = 1.0           # fold: out[j] = in[64+j]
        PPa[j, 64 + j] = 1.0           # dup:  out[j] = in[j]
        PPa[j, 128 + j] = 1.0          # dup:  out[64+j] = in[j]
    shared = {"WcT": WcT, "bcT": bcT, "W1T": W1T, "W2P": W2P, "b1c": b1c,
              "s1c": s1c, "consts": consts, "PP": PPa}
    in_maps = []
    for i in range(NCORES):
        s = slice(i * BL, (i + 1) * BL)
        in_maps.append({"hT": np.ascontiguousarray(hT[:, s]),
                        "cT": np.ascontiguousarray(cT[:, s]), **shared})
    return in_maps


def postprocess(results, gamma3, beta3, W2, b2):
    """Host-side BN3 + final affine from per-core raw' and y' stats."""
    W2 = np.asarray(W2, np.float64)
    s2 = W2.sum(1)                      # [2]
    b2 = np.asarray(b2, np.float64)
    n_row = T * 256                     # samples per ystats row (J subsample)

    A = [float(np.asarray(r["scout"])[0, 0]) for r in results]
    tot_n = 0.0
    tot_sum = 0.0
    tot_sumsq = 0.0
    for i, r in enumerate(results):
        st = np.asarray(r["yst"], np.float64)     # [114, 2] mean/var of y'
        rows = np.r_[0:50, 64:114]
        mean = st[rows, 0]
        var = st[rows, 1]
        tot_sum += A[i] * mean.sum() * n_row
        tot_sumsq += A[i] * A[i] * (var + mean * mean).sum() * n_row
        tot_n += len(rows) * n_row
    m3 = tot_sum / tot_n
    v3 = tot_sumsq / tot_n - m3 * m3
    a3 = float(gamma3) / np.sqrt(v3 + EPS)
    shift = (float(beta3) - a3 * m3) * s2 + b2    # [2]

    out = np.empty((B, T, 2), np.float32)
    for i, r in enumerate(results):
        raw = np.asarray(r["raw"]).astype(np.float64)  # [12,100,512] bf16
        sel = np.r_[0:4, 32:36, 64:68, 96:100]
        rr = raw[:, sel, :].reshape(T, 4, 2, 2, 512)  # [t, J, half, k, col]
        val = a3 * A[i] * rr + shift[None, None, None, :, None]
        # b_local = J*1024 + half*512 + col
        val = val.transpose(0, 3, 1, 2, 4).reshape(T, 2, BL)  # [t,k,b]
        out[i * BL:(i + 1) * BL] = val.transpose(2, 0, 1).astype(np.float32)
    return out


def kernel(h, c, W_ih, W_hh, b_ih, b_hh, gamma1, beta1, gamma2, beta2,
           gamma3, beta3, W1, b1, W2, b2):
    in_maps = prep_in_maps(h, c, W_ih, W_hh, b_ih, b_hh, gamma1, beta1,
                           gamma2, beta2, gamma3, beta3, W1, b1, W2, b2)
    nc = _get_nc()
    res = run_bass_kernel_spmd(nc, in_maps, list(range(NCORES)))
    return postprocess(res.results, gamma3, beta3, W2, b2)
